# revision 1
# baseline (speedup 1.0000x reference)
"""MoE (MiniMax decoder MLP) Trainium2 kernel — expert-parallel across 8 NeuronCores.

Strategy (per the expert-parallel sharding hint):
  - Host computes the router (softmax + top-2 + renormalize) — this IS the
    sharding decision — and dispatches each token's activation row to the
    core(s) owning its selected expert(s).
  - Core e holds expert e's weights [H,I]/[H,I]/[I,H] and computes
    silu(x @ Wg) * (x @ Wu) @ Wd for its routed tokens (padded to a common
    capacity C), scaling rows by the renormalized combine weight on-device.
  - Host scatter-adds the per-expert outputs back into the full [T, H] output.

Compute is done in bf16 on the TensorEngine (fp32 PSUM accumulation).
"""

import os
import sys

import numpy as np

_EXTRA_PATHS = [
    "/root/.axon_site",
    "/root/.axon_site/_ro/trn_rl_repo",
    "/root/.axon_site/_ro/pypackages",
    "/opt/trn_rl_repo",
    "/opt/pypackages",
]
try:
    import concourse.bass  # noqa: F401
except ImportError:  # pragma: no cover
    sys.path[:0] = [p for p in _EXTRA_PATHS if p not in sys.path]

import ml_dtypes

B, S, H = 4, 2048, 2048
I = 1408  # expert intermediate size
E = 8  # num experts
K = 2  # experts per token
N_CORES = 8

_NC_CACHE = {}


def _build_nc(C):
    """Build + compile the per-core expert MLP program for capacity C tokens."""
    import concourse.mybir as mybir
    import concourse.tile as tile
    from concourse import bacc

    fp32 = mybir.dt.float32
    bf16 = mybir.dt.bfloat16
    mult = mybir.AluOpType.mult
    silu_fn = mybir.ActivationFunctionType.Silu

    assert C % 128 == 0
    KT = H // 128  # 16 contraction tiles over H
    IT = I // 128  # 11 tiles over I
    HW = H // 512  # 4 output windows over H
    MT = C // 128  # token m-tiles

    # Token windows of up to 512.
    windows = []
    o = 0
    while o < C:
        w = min(512, C - o)
        windows.append((o, w))
        o += w

    nc = bacc.Bacc("TRN2", target_bir_lowering=False, debug=False, num_devices=N_CORES)

    xt = nc.dram_tensor("xt", [H, C], bf16, kind="ExternalInput")
    wg = nc.dram_tensor("wg", [H, I], bf16, kind="ExternalInput")
    wu = nc.dram_tensor("wu", [H, I], bf16, kind="ExternalInput")
    wd = nc.dram_tensor("wd", [I, H], bf16, kind="ExternalInput")
    cw = nc.dram_tensor("cw", [C, 1], fp32, kind="ExternalInput")
    out = nc.dram_tensor("out", [C, H], fp32, kind="ExternalOutput")

    with tile.TileContext(nc) as tc:
        with (
            tc.tile_pool(name="wpool", bufs=1) as wpool,
            tc.tile_pool(name="xpool", bufs=2) as xpool,
            tc.tile_pool(name="gpool", bufs=2) as gpool,
            tc.tile_pool(name="spool", bufs=2) as spool,
            tc.tile_pool(name="opool", bufs=3) as opool,
            tc.tile_pool(name="cwpool", bufs=1) as cwpool,
            tc.tile_pool(name="pgp", bufs=2, space="PSUM") as pgp,
            tc.tile_pool(name="pup", bufs=2, space="PSUM") as pup,
            tc.tile_pool(name="pop", bufs=3, space="PSUM") as pop,
        ):
            # Combine weights: [C,1] -> [128, MT], element [p, n] = cw[n*128+p].
            cw_sb = cwpool.tile([128, MT], fp32, name="cw_sb", tag="cw_sb")
            nc.sync.dma_start(
                cw_sb[:], cw.ap().rearrange("(n p) o -> p (n o)", p=128)
            )

            # Resident expert weights (bf16): 16+16+11 tiles.
            wg_sb, wu_sb, wd_sb = [], [], []
            for k in range(KT):
                t = wpool.tile([128, I], bf16, name=f"wg_sb{k}", tag=f"wg_sb{k}")
                nc.sync.dma_start(t[:], wg.ap()[k * 128 : (k + 1) * 128, :])
                wg_sb.append(t)
            for k in range(KT):
                t = wpool.tile([128, I], bf16, name=f"wu_sb{k}", tag=f"wu_sb{k}")
                nc.sync.dma_start(t[:], wu.ap()[k * 128 : (k + 1) * 128, :])
                wu_sb.append(t)
            for i in range(IT):
                t = wpool.tile([128, H], bf16, name=f"wd_sb{i}", tag=f"wd_sb{i}")
                nc.sync.dma_start(t[:], wd.ap()[i * 128 : (i + 1) * 128, :])
                wd_sb.append(t)

            for o, W in windows:
                # Stream this window's activations: xT [H, W] as 16 k-tiles.
                xt_sb = []
                for k in range(KT):
                    t = xpool.tile([128, 512], bf16, name=f"xt_sb{k}", tag=f"xt_sb{k}")
                    nc.sync.dma_start(
                        t[:, :W], xt.ap()[k * 128 : (k + 1) * 128, o : o + W]
                    )
                    xt_sb.append(t)

                # gated^T tiles [I-tile, W] in bf16 for the down matmul.
                gated = []
                for i in range(IT):
                    pg = pgp.tile([128, 512], fp32, name="pg", tag="pg")
                    pu = pup.tile([128, 512], fp32, name="pu", tag="pu")
                    for k in range(KT):
                        nc.tensor.matmul(
                            pg[:, :W],
                            wg_sb[k][:, i * 128 : (i + 1) * 128],
                            xt_sb[k][:, :W],
                            start=(k == 0),
                            stop=(k == KT - 1),
                        )
                    for k in range(KT):
                        nc.tensor.matmul(
                            pu[:, :W],
                            wu_sb[k][:, i * 128 : (i + 1) * 128],
                            xt_sb[k][:, :W],
                            start=(k == 0),
                            stop=(k == KT - 1),
                        )
                    act = spool.tile([128, 512], fp32, name="act", tag="act")
                    nc.scalar.activation(act[:, :W], pg[:, :W], silu_fn)
                    g = gpool.tile([128, 512], bf16, name=f"g{i}", tag=f"g{i}")
                    nc.vector.tensor_tensor(g[:, :W], act[:, :W], pu[:, :W], mult)
                    gated.append(g)

                # Down-proj: out[tokens, H] accumulated over I, then scale by
                # the per-token combine weight.
                for m in range(W // 128):
                    mg = o // 128 + m
                    for h in range(HW):
                        po = pop.tile([128, 512], fp32, name="po", tag="po")
                        for i in range(IT):
                            nc.tensor.matmul(
                                po[:],
                                gated[i][:, m * 128 : (m + 1) * 128],
                                wd_sb[i][:, h * 512 : (h + 1) * 512],
                                start=(i == 0),
                                stop=(i == IT - 1),
                            )
                        ob = opool.tile([128, 512], fp32, name="ob", tag="ob")
                        nc.vector.tensor_scalar_mul(ob[:], po[:], cw_sb[:, mg : mg + 1])
                        nc.sync.dma_start(
                            out.ap()[
                                o + m * 128 : o + (m + 1) * 128,
                                h * 512 : (h + 1) * 512,
                            ],
                            ob[:],
                        )

    nc.compile()
    return nc


def kernel(
    hidden_states: np.ndarray,
    gate_w: np.ndarray,
    w_gate: np.ndarray,
    w_up: np.ndarray,
    w_down: np.ndarray,
) -> np.ndarray:
    from concourse.bass_utils import run_bass_kernel_spmd

    x = np.asarray(hidden_states, dtype=np.float32).reshape(-1, H)
    gate_w = np.asarray(gate_w, dtype=np.float32)
    w_gate = np.asarray(w_gate, dtype=np.float32)
    w_up = np.asarray(w_up, dtype=np.float32)
    w_down = np.asarray(w_down, dtype=np.float32)
    T = x.shape[0]

    # Router (the sharding decision): softmax over experts, top-2, renormalize.
    logits = x @ gate_w.T
    logits -= logits.max(axis=-1, keepdims=True)
    ex = np.exp(logits)
    probs = ex / ex.sum(axis=-1, keepdims=True)
    topk_i = np.argpartition(-probs, K - 1, axis=-1)[:, :K]  # [T, K]
    topk_w = np.take_along_axis(probs, topk_i, axis=-1)
    denom = topk_w.sum(axis=-1)  # [T]

    sels, cws = [], []
    for e in range(E):
        sel = np.nonzero((topk_i == e).any(axis=1))[0]
        sels.append(sel)
        cws.append(probs[sel, e] / denom[sel])

    max_count = max(len(s) for s in sels)
    C = max(128, -(-max_count // 128) * 128)

    if C not in _NC_CACHE:
        _NC_CACHE[C] = _build_nc(C)
    nc = _NC_CACHE[C]

    # Dispatch: gather each expert's tokens (transposed, bf16) + weights.
    xt_full = np.ascontiguousarray(x.T.astype(ml_dtypes.bfloat16))  # [H, T]
    wg_b = w_gate.astype(ml_dtypes.bfloat16)
    wu_b = w_up.astype(ml_dtypes.bfloat16)
    wd_b = w_down.astype(ml_dtypes.bfloat16)

    in_maps = []
    for e in range(E):
        sel = sels[e]
        xt_e = np.zeros((H, C), dtype=ml_dtypes.bfloat16)
        xt_e[:, : len(sel)] = xt_full[:, sel]
        cw_e = np.zeros((C, 1), dtype=np.float32)
        cw_e[: len(sel), 0] = cws[e]
        in_maps.append(
            {"xt": xt_e, "wg": wg_b[e], "wu": wu_b[e], "wd": wd_b[e], "cw": cw_e}
        )

    trace = bool(os.environ.get("BASS_MOE_TRACE"))
    res = run_bass_kernel_spmd(
        nc, in_maps, core_ids=list(range(N_CORES)), trace=trace
    )
    if trace and res.exec_time_ns is not None:
        print(f"HW exec time: {res.exec_time_ns} ns")

    # Combine: scatter-add each expert's (already weight-scaled) rows.
    out_full = np.zeros((T, H), dtype=np.float32)
    for e in range(E):
        sel = sels[e]
        out_full[sel] += res.results[e]["out"][: len(sel)]
    return out_full.reshape(B, S, H)


# revision 3
# speedup vs baseline: 1.0268x; 1.0268x over previous
"""MoE (MiniMax decoder MLP) Trainium2 kernel — expert-parallel across 8 NeuronCores.

Strategy (per the expert-parallel sharding hint):
  - Host computes the router (softmax + top-2 + renormalize) — this IS the
    sharding decision — and dispatches each token's activation row to the
    core(s) owning its selected expert(s).
  - Core e holds expert e's weights [H,I]/[H,I]/[I,H] and computes
    silu(x @ Wg) * (x @ Wu) @ Wd for its routed tokens (padded to a common
    capacity C), scaling rows by the renormalized combine weight on-device.
  - Host scatter-adds the per-expert outputs back into the full [T, H] output.

Compute is done in bf16 on the TensorEngine (fp32 PSUM accumulation).
"""

import os
import sys

import numpy as np

_EXTRA_PATHS = [
    "/root/.axon_site",
    "/root/.axon_site/_ro/trn_rl_repo",
    "/root/.axon_site/_ro/pypackages",
    "/opt/trn_rl_repo",
    "/opt/pypackages",
]
try:
    import concourse.bass  # noqa: F401
except ImportError:  # pragma: no cover
    sys.path[:0] = [p for p in _EXTRA_PATHS if p not in sys.path]

import ml_dtypes

B, S, H = 4, 2048, 2048
I = 1408  # expert intermediate size
E = 8  # num experts
K = 2  # experts per token
N_CORES = 8

_NC_CACHE = {}


def _build_nc(C):
    """Build + compile the per-core expert MLP program for capacity C tokens."""
    import concourse.mybir as mybir
    import concourse.tile as tile
    from concourse import bacc

    fp32 = mybir.dt.float32
    bf16 = mybir.dt.bfloat16
    mult = mybir.AluOpType.mult
    silu_fn = mybir.ActivationFunctionType.Silu

    assert C % 128 == 0
    KT = H // 128  # 16 contraction tiles over H
    IT = I // 128  # 11 tiles over I
    HW = H // 512  # 4 output windows over H
    MT = C // 128  # token m-tiles

    # Token windows: a small 128-token window first (so the PE can start
    # while the expert weights are still streaming in), then chunks of <=512.
    windows = [(0, 128)]
    o = 128
    while o < C:
        w = min(512, C - o)
        if (C - o) % 512 != 0 and C - o > 512:
            w = (C - o) % 512
        windows.append((o, w))
        o += w

    nc = bacc.Bacc("TRN2", target_bir_lowering=False, debug=False, num_devices=N_CORES)

    xt = nc.dram_tensor("xt", [H, C], bf16, kind="ExternalInput")
    wg = nc.dram_tensor("wg", [H, I], bf16, kind="ExternalInput")
    wu = nc.dram_tensor("wu", [H, I], bf16, kind="ExternalInput")
    wd = nc.dram_tensor("wd", [I, H], bf16, kind="ExternalInput")
    cw = nc.dram_tensor("cw", [C, 1], fp32, kind="ExternalInput")
    out = nc.dram_tensor("out", [C, H], fp32, kind="ExternalOutput")

    with tile.TileContext(nc) as tc:
        with (
            tc.tile_pool(name="wpool", bufs=1) as wpool,
            tc.tile_pool(name="xpool", bufs=2) as xpool,
            tc.tile_pool(name="gpool", bufs=2) as gpool,
            tc.tile_pool(name="spool", bufs=2) as spool,
            tc.tile_pool(name="opool", bufs=3) as opool,
            tc.tile_pool(name="cwpool", bufs=1) as cwpool,
            tc.tile_pool(name="w0pool", bufs=1) as w0pool,
            tc.tile_pool(name="pgp", bufs=2, space="PSUM") as pgp,
            tc.tile_pool(name="pup", bufs=2, space="PSUM") as pup,
            tc.tile_pool(name="pop", bufs=3, space="PSUM") as pop,
        ):
            # Window-0 activations first: the PE starts on these while the
            # bulk of the weights is still streaming in.
            o0, W0 = windows[0]
            xt0_sb = []
            for k in range(KT):
                t = xpool.tile([128, 512], bf16, name=f"xt_sb{k}", tag=f"xt_sb{k}")
                nc.sync.dma_start(t[:, :W0], xt.ap()[k * 128 : (k + 1) * 128, o0 : o0 + W0])
                xt0_sb.append(t)

            # Combine weights: [C,1] -> [128, MT], element [p, n] = cw[n*128+p].
            cw_sb = cwpool.tile([128, MT], fp32, name="cw_sb", tag="cw_sb")
            nc.sync.dma_start(
                cw_sb[:], cw.ap().rearrange("(n p) o -> p (n o)", p=128)
            )

            # Resident expert weights (bf16): 16+16+11 tiles. Order matters:
            # wg feeds the first PE sweep, wu the second, wd only the deferred
            # down-matmuls (a full window later).
            wg_sb, wu_sb, wd_sb = [], [], []
            for k in range(KT):
                t = wpool.tile([128, I], bf16, name=f"wg_sb{k}", tag=f"wg_sb{k}")
                nc.sync.dma_start(t[:], wg.ap()[k * 128 : (k + 1) * 128, :])
                wg_sb.append(t)
            for k in range(KT):
                t = wpool.tile([128, I], bf16, name=f"wu_sb{k}", tag=f"wu_sb{k}")
                nc.sync.dma_start(t[:], wu.ap()[k * 128 : (k + 1) * 128, :])
                wu_sb.append(t)
            for i in range(IT):
                t = wpool.tile([128, H], bf16, name=f"wd_sb{i}", tag=f"wd_sb{i}")
                nc.sync.dma_start(t[:], wd.ap()[i * 128 : (i + 1) * 128, :])
                wd_sb.append(t)

            def emit_matmul1(xt_sb, W, split):
                """silu(x@Wg) * (x@Wu) for one token window -> gated^T tiles.

                split=True emits the full Wg sweep before the Wu sweep
                (start-up: lets the PE run gate matmuls while wu still loads).
                """
                gated = []
                if split:
                    acts = []
                    for i in range(IT):
                        pg = pgp.tile([128, 512], fp32, name="pg", tag="pg")
                        for k in range(KT):
                            nc.tensor.matmul(
                                pg[:, :W],
                                wg_sb[k][:, i * 128 : (i + 1) * 128],
                                xt_sb[k][:, :W],
                                start=(k == 0),
                                stop=(k == KT - 1),
                            )
                        act = w0pool.tile([128, 128], fp32, name=f"w0act{i}", tag=f"w0act{i}")
                        nc.scalar.activation(act[:, :W], pg[:, :W], silu_fn)
                        acts.append(act)
                    for i in range(IT):
                        pu = pup.tile([128, 512], fp32, name="pu", tag="pu")
                        for k in range(KT):
                            nc.tensor.matmul(
                                pu[:, :W],
                                wu_sb[k][:, i * 128 : (i + 1) * 128],
                                xt_sb[k][:, :W],
                                start=(k == 0),
                                stop=(k == KT - 1),
                            )
                        g = gpool.tile([128, 512], bf16, name=f"g{i}", tag=f"g{i}")
                        nc.vector.tensor_tensor(g[:, :W], acts[i][:, :W], pu[:, :W], mult)
                        gated.append(g)
                else:
                    for i in range(IT):
                        pg = pgp.tile([128, 512], fp32, name="pg", tag="pg")
                        pu = pup.tile([128, 512], fp32, name="pu", tag="pu")
                        for k in range(KT):
                            nc.tensor.matmul(
                                pg[:, :W],
                                wg_sb[k][:, i * 128 : (i + 1) * 128],
                                xt_sb[k][:, :W],
                                start=(k == 0),
                                stop=(k == KT - 1),
                            )
                        for k in range(KT):
                            nc.tensor.matmul(
                                pu[:, :W],
                                wu_sb[k][:, i * 128 : (i + 1) * 128],
                                xt_sb[k][:, :W],
                                start=(k == 0),
                                stop=(k == KT - 1),
                            )
                        act = spool.tile([128, 512], fp32, name="act", tag="act")
                        nc.scalar.activation(act[:, :W], pg[:, :W], silu_fn)
                        g = gpool.tile([128, 512], bf16, name=f"g{i}", tag=f"g{i}")
                        nc.vector.tensor_tensor(g[:, :W], act[:, :W], pu[:, :W], mult)
                        gated.append(g)
                return gated

            def emit_matmul2(o, W, gated):
                # Down-proj: out[tokens, H] accumulated over I, then scale by
                # the per-token combine weight.
                for m in range(W // 128):
                    mg = o // 128 + m
                    for h in range(HW):
                        po = pop.tile([128, 512], fp32, name="po", tag="po")
                        for i in range(IT):
                            nc.tensor.matmul(
                                po[:],
                                gated[i][:, m * 128 : (m + 1) * 128],
                                wd_sb[i][:, h * 512 : (h + 1) * 512],
                                start=(i == 0),
                                stop=(i == IT - 1),
                            )
                        ob = opool.tile([128, 512], fp32, name="ob", tag="ob")
                        nc.vector.tensor_scalar_mul(ob[:], po[:], cw_sb[:, mg : mg + 1])
                        nc.sync.dma_start(
                            out.ap()[
                                o + m * 128 : o + (m + 1) * 128,
                                h * 512 : (h + 1) * 512,
                            ],
                            ob[:],
                        )

            # Window pipeline: matmul2 of window t is emitted after matmul1 of
            # window t+1 (gpool bufs=2 keeps both windows' gated tiles live),
            # so the start-up down-matmuls don't stall on the wd load.
            pending = None
            for wi, (o, W) in enumerate(windows):
                if wi == 0:
                    xt_sb = xt0_sb
                else:
                    xt_sb = []
                    for k in range(KT):
                        t = xpool.tile([128, 512], bf16, name=f"xt_sb{k}", tag=f"xt_sb{k}")
                        nc.sync.dma_start(
                            t[:, :W], xt.ap()[k * 128 : (k + 1) * 128, o : o + W]
                        )
                        xt_sb.append(t)
                gated = emit_matmul1(xt_sb, W, split=(wi == 0))
                if pending is not None:
                    emit_matmul2(*pending)
                pending = (o, W, gated)
            emit_matmul2(*pending)

    nc.compile()
    return nc


def kernel(
    hidden_states: np.ndarray,
    gate_w: np.ndarray,
    w_gate: np.ndarray,
    w_up: np.ndarray,
    w_down: np.ndarray,
) -> np.ndarray:
    from concourse.bass_utils import run_bass_kernel_spmd

    x = np.asarray(hidden_states, dtype=np.float32).reshape(-1, H)
    gate_w = np.asarray(gate_w, dtype=np.float32)
    w_gate = np.asarray(w_gate, dtype=np.float32)
    w_up = np.asarray(w_up, dtype=np.float32)
    w_down = np.asarray(w_down, dtype=np.float32)
    T = x.shape[0]

    # Router (the sharding decision): softmax over experts, top-2, renormalize.
    logits = x @ gate_w.T
    logits -= logits.max(axis=-1, keepdims=True)
    ex = np.exp(logits)
    probs = ex / ex.sum(axis=-1, keepdims=True)
    topk_i = np.argpartition(-probs, K - 1, axis=-1)[:, :K]  # [T, K]
    topk_w = np.take_along_axis(probs, topk_i, axis=-1)
    denom = topk_w.sum(axis=-1)  # [T]

    sels, cws = [], []
    for e in range(E):
        sel = np.nonzero((topk_i == e).any(axis=1))[0]
        sels.append(sel)
        cws.append(probs[sel, e] / denom[sel])

    max_count = max(len(s) for s in sels)
    C = max(128, -(-max_count // 128) * 128)

    if C not in _NC_CACHE:
        _NC_CACHE[C] = _build_nc(C)
    nc = _NC_CACHE[C]

    # Dispatch: gather each expert's tokens (transposed, bf16) + weights.
    xt_full = np.ascontiguousarray(x.T.astype(ml_dtypes.bfloat16))  # [H, T]
    wg_b = w_gate.astype(ml_dtypes.bfloat16)
    wu_b = w_up.astype(ml_dtypes.bfloat16)
    wd_b = w_down.astype(ml_dtypes.bfloat16)

    in_maps = []
    for e in range(E):
        sel = sels[e]
        xt_e = np.zeros((H, C), dtype=ml_dtypes.bfloat16)
        xt_e[:, : len(sel)] = xt_full[:, sel]
        cw_e = np.zeros((C, 1), dtype=np.float32)
        cw_e[: len(sel), 0] = cws[e]
        in_maps.append(
            {"xt": xt_e, "wg": wg_b[e], "wu": wu_b[e], "wd": wd_b[e], "cw": cw_e}
        )

    trace = bool(os.environ.get("BASS_MOE_TRACE"))
    res = run_bass_kernel_spmd(
        nc, in_maps, core_ids=list(range(N_CORES)), trace=trace
    )
    if trace and res.exec_time_ns is not None:
        print(f"HW exec time: {res.exec_time_ns} ns")

    # Combine: scatter-add each expert's (already weight-scaled) rows.
    out_full = np.zeros((T, H), dtype=np.float32)
    for e in range(E):
        sel = sels[e]
        out_full[sel] += res.results[e]["out"][: len(sel)]
    return out_full.reshape(B, S, H)


# revision 4
# speedup vs baseline: 1.0876x; 1.0591x over previous
"""MoE (MiniMax decoder MLP) Trainium2 kernel — expert-parallel across 8 NeuronCores.

Strategy (per the expert-parallel sharding hint):
  - Host computes the router (softmax + top-2 + renormalize) — this IS the
    sharding decision — and dispatches each token's activation row to the
    core(s) owning its selected expert(s).
  - Core e holds expert e's weights [H,I]/[H,I]/[I,H] and computes
    silu(x @ Wg) * (x @ Wu) @ Wd for its routed tokens (padded to a common
    capacity C), scaling rows by the renormalized combine weight on-device.
  - Host scatter-adds the per-expert outputs back into the full [T, H] output.

Compute is done in bf16 on the TensorEngine (fp32 PSUM accumulation).

Gate/up weights are host-swizzled into an i-block-major SBUF image so each
128-wide intermediate block is one large contiguous DMA; gate/up blocks are
interleaved so the PE's first accumulation groups start ~6us into the kernel
instead of waiting for the full 11.5MB weight load. Activation (xt) DMAs are
issued from the scalar queue (the second HWDGE engine) so they don't serialize
behind the weight DMAs on the sync queue.
"""

import os
import sys

import numpy as np

_EXTRA_PATHS = [
    "/root/.axon_site",
    "/root/.axon_site/_ro/trn_rl_repo",
    "/root/.axon_site/_ro/pypackages",
    "/opt/trn_rl_repo",
    "/opt/pypackages",
]
try:
    import concourse.bass  # noqa: F401
except ImportError:  # pragma: no cover
    sys.path[:0] = [p for p in _EXTRA_PATHS if p not in sys.path]

import ml_dtypes

B, S, H = 4, 2048, 2048
I = 1408  # expert intermediate size
E = 8  # num experts
K = 2  # experts per token
N_CORES = 8

KT = H // 128  # 16 contraction tiles over H
IT = I // 128  # 11 tiles over I
HW = H // 512  # 4 output windows over H

_NC_CACHE = {}


def _windows(C):
    # Full 512-token windows, remainder (multiple of 128) last.
    ws = []
    o = 0
    while o < C:
        w = min(512, C - o)
        ws.append((o, w))
        o += w
    return ws


def _build_nc(C):
    """Build + compile the per-core expert MLP program for capacity C tokens."""
    import concourse.mybir as mybir
    import concourse.tile as tile
    from concourse import bacc

    fp32 = mybir.dt.float32
    bf16 = mybir.dt.bfloat16
    mult = mybir.AluOpType.mult
    silu_fn = mybir.ActivationFunctionType.Silu

    assert C % 128 == 0
    MT = C // 128  # token m-tiles
    windows = _windows(C)

    nc = bacc.Bacc("TRN2", target_bir_lowering=False, debug=False, num_devices=N_CORES)

    # wg/wu arrive pre-swizzled: [128, IT*KT*128], free offset (i*KT + k)*128
    # holds wg[k*128+p, i*128+c] at column c. wd is its natural [I, H] layout.
    xt = nc.dram_tensor("xt", [H, C], bf16, kind="ExternalInput")
    wg = nc.dram_tensor("wg", [128, IT * KT * 128], bf16, kind="ExternalInput")
    wu = nc.dram_tensor("wu", [128, IT * KT * 128], bf16, kind="ExternalInput")
    wd = nc.dram_tensor("wd", [I, H], bf16, kind="ExternalInput")
    cw = nc.dram_tensor("cw", [C, 1], fp32, kind="ExternalInput")
    out = nc.dram_tensor("out", [C, H], fp32, kind="ExternalOutput")

    with tile.TileContext(nc) as tc:
        with (
            tc.tile_pool(name="wpool", bufs=1) as wpool,
            tc.tile_pool(name="xpool", bufs=2) as xpool,
            tc.tile_pool(name="gpool", bufs=2) as gpool,
            tc.tile_pool(name="spool", bufs=2) as spool,
            tc.tile_pool(name="opool", bufs=3) as opool,
            tc.tile_pool(name="cwpool", bufs=1) as cwpool,
            tc.tile_pool(name="pgp", bufs=2, space="PSUM") as pgp,
            tc.tile_pool(name="pup", bufs=2, space="PSUM") as pup,
            tc.tile_pool(name="pop", bufs=3, space="PSUM") as pop,
        ):
            # Combine weights: [C,1] -> [128, MT], element [p, n] = cw[n*128+p].
            cw_sb = cwpool.tile([128, MT], fp32, name="cw_sb", tag="cw_sb")
            nc.scalar.dma_start(
                cw_sb[:], cw.ap().rearrange("(n p) o -> p (n o)", p=128)
            )

            def dma_xt_window(o, W):
                xt_sb = []
                for k in range(KT):
                    t = xpool.tile([128, 512], bf16, name=f"xt_sb{k}", tag=f"xt_sb{k}")
                    nc.scalar.dma_start(
                        t[:, :W], xt.ap()[k * 128 : (k + 1) * 128, o : o + W]
                    )
                    xt_sb.append(t)
                return xt_sb

            # Window-0 activations issue first on the scalar queue.
            xt0_sb = dma_xt_window(*windows[0])

            # Expert weights (bf16), on the sync queue: interleaved gate/up
            # i-blocks (0.5MB each), then the down-proj blocks.
            wg_sb = wpool.tile([128, IT * KT * 128], bf16, name="wg_sb", tag="wg_sb")
            wu_sb = wpool.tile([128, IT * KT * 128], bf16, name="wu_sb", tag="wu_sb")
            wd_sb = wpool.tile([128, IT * H], bf16, name="wd_sb", tag="wd_sb")
            blk = KT * 128
            for i in range(IT):
                nc.sync.dma_start(
                    wg_sb[:, i * blk : (i + 1) * blk],
                    wg.ap()[:, i * blk : (i + 1) * blk],
                )
                nc.sync.dma_start(
                    wu_sb[:, i * blk : (i + 1) * blk],
                    wu.ap()[:, i * blk : (i + 1) * blk],
                )
            for i in range(IT):
                nc.sync.dma_start(
                    wd_sb[:, i * H : (i + 1) * H],
                    wd.ap()[i * 128 : (i + 1) * 128, :],
                )

            def emit_matmul1(xt_sb, W):
                """silu(x@Wg) * (x@Wu) for one token window -> gated^T tiles."""
                gated = []
                for i in range(IT):
                    pg = pgp.tile([128, 512], fp32, name="pg", tag="pg")
                    pu = pup.tile([128, 512], fp32, name="pu", tag="pu")
                    for k in range(KT):
                        nc.tensor.matmul(
                            pg[:, :W],
                            wg_sb[:, i * blk + k * 128 : i * blk + (k + 1) * 128],
                            xt_sb[k][:, :W],
                            start=(k == 0),
                            stop=(k == KT - 1),
                        )
                    for k in range(KT):
                        nc.tensor.matmul(
                            pu[:, :W],
                            wu_sb[:, i * blk + k * 128 : i * blk + (k + 1) * 128],
                            xt_sb[k][:, :W],
                            start=(k == 0),
                            stop=(k == KT - 1),
                        )
                    act = spool.tile([128, 512], fp32, name="act", tag="act")
                    nc.scalar.activation(act[:, :W], pg[:, :W], silu_fn)
                    g = gpool.tile([128, 512], bf16, name=f"g{i}", tag=f"g{i}")
                    nc.vector.tensor_tensor(g[:, :W], act[:, :W], pu[:, :W], mult)
                    gated.append(g)
                return gated

            def emit_matmul2(o, W, gated):
                # Down-proj: out[tokens, H] accumulated over I, then scale by
                # the per-token combine weight.
                for m in range(W // 128):
                    mg = o // 128 + m
                    for h in range(HW):
                        po = pop.tile([128, 512], fp32, name="po", tag="po")
                        for i in range(IT):
                            nc.tensor.matmul(
                                po[:],
                                gated[i][:, m * 128 : (m + 1) * 128],
                                wd_sb[:, i * H + h * 512 : i * H + (h + 1) * 512],
                                start=(i == 0),
                                stop=(i == IT - 1),
                            )
                        ob = opool.tile([128, 512], fp32, name="ob", tag="ob")
                        nc.vector.tensor_scalar_mul(ob[:], po[:], cw_sb[:, mg : mg + 1])
                        nc.sync.dma_start(
                            out.ap()[
                                o + m * 128 : o + (m + 1) * 128,
                                h * 512 : (h + 1) * 512,
                            ],
                            ob[:],
                        )

            # Window pipeline: matmul2 of window t is emitted after matmul1 of
            # window t+1 (gpool bufs=2 keeps both windows' gated tiles live),
            # so the start-up down-matmuls don't stall on the wd load.
            pending = None
            for wi, (o, W) in enumerate(windows):
                xt_sb = xt0_sb if wi == 0 else dma_xt_window(o, W)
                gated = emit_matmul1(xt_sb, W)
                if pending is not None:
                    emit_matmul2(*pending)
                pending = (o, W, gated)
            emit_matmul2(*pending)

    nc.compile()
    return nc


def kernel(
    hidden_states: np.ndarray,
    gate_w: np.ndarray,
    w_gate: np.ndarray,
    w_up: np.ndarray,
    w_down: np.ndarray,
) -> np.ndarray:
    from concourse.bass_utils import run_bass_kernel_spmd

    x = np.asarray(hidden_states, dtype=np.float32).reshape(-1, H)
    gate_w = np.asarray(gate_w, dtype=np.float32)
    w_gate = np.asarray(w_gate, dtype=np.float32)
    w_up = np.asarray(w_up, dtype=np.float32)
    w_down = np.asarray(w_down, dtype=np.float32)
    T = x.shape[0]

    # Router (the sharding decision): softmax over experts, top-2, renormalize.
    logits = x @ gate_w.T
    logits -= logits.max(axis=-1, keepdims=True)
    ex = np.exp(logits)
    probs = ex / ex.sum(axis=-1, keepdims=True)
    topk_i = np.argpartition(-probs, K - 1, axis=-1)[:, :K]  # [T, K]
    topk_w = np.take_along_axis(probs, topk_i, axis=-1)
    denom = topk_w.sum(axis=-1)  # [T]

    sels, cws = [], []
    for e in range(E):
        sel = np.nonzero((topk_i == e).any(axis=1))[0]
        sels.append(sel)
        cws.append(probs[sel, e] / denom[sel])

    max_count = max(len(s) for s in sels)
    C = max(128, -(-max_count // 128) * 128)

    if C not in _NC_CACHE:
        _NC_CACHE[C] = _build_nc(C)
    nc = _NC_CACHE[C]

    # Dispatch: gather each expert's tokens (transposed, bf16) + weights.
    xt_full = np.ascontiguousarray(x.T.astype(ml_dtypes.bfloat16))  # [H, T]

    def swz(w):  # [H, I] -> [128, IT*KT*128] i-block-major SBUF image
        return np.ascontiguousarray(
            w.astype(ml_dtypes.bfloat16)
            .reshape(KT, 128, IT, 128)
            .transpose(1, 2, 0, 3)
            .reshape(128, IT * KT * 128)
        )

    in_maps = []
    for e in range(E):
        sel = sels[e]
        xt_e = np.zeros((H, C), dtype=ml_dtypes.bfloat16)
        xt_e[:, : len(sel)] = xt_full[:, sel]
        cw_e = np.zeros((C, 1), dtype=np.float32)
        cw_e[: len(sel), 0] = cws[e]
        in_maps.append(
            {
                "xt": xt_e,
                "wg": swz(w_gate[e]),
                "wu": swz(w_up[e]),
                "wd": w_down[e].astype(ml_dtypes.bfloat16),
                "cw": cw_e,
            }
        )

    trace = bool(os.environ.get("BASS_MOE_TRACE"))
    res = run_bass_kernel_spmd(
        nc, in_maps, core_ids=list(range(N_CORES)), trace=trace
    )
    if trace and res.exec_time_ns is not None:
        print(f"HW exec time: {res.exec_time_ns} ns")

    # Combine: scatter-add each expert's (already weight-scaled) rows.
    out_full = np.zeros((T, H), dtype=np.float32)
    for e in range(E):
        sel = sels[e]
        out_full[sel] += res.results[e]["out"][: len(sel)]
    return out_full.reshape(B, S, H)
